# revision 6
# baseline (speedup 1.0000x reference)
"""AttentionPool Trainium2 kernel: 8-core data-parallel Bass/Tile implementation.

Reference computation (per batch b of 32, S=2048, D=1024):
    xn = LayerNorm(x[b])                      # over D, eps 1e-5
    h = tanh(xn @ W1 + b1)
    scores = h @ W2 + b2                      # [S]
    w = softmax(scores)
    out[b] = sum_s w[s] * x[b, s, :]

v2 design: batch axis sharded over 8 cores (4 batches each). The core's
work is a single software pipeline over 16 "quarters" (512 seq rows
each; 4 per batch), with stages offset so every engine queue stays
dependency-ready:

    step t emits:  load(t+4)       x quarter, f32, gpsimd SWDGE ring
                   transpose(t+1)  4x SBUF->SBUF DMA-xbar transposes
                   p3(t)           mm1 fp8 DoubleRow + tanh + scores
                   pool(t-1)       bf16 pooling matmuls (PSUM accum)
                   norm(t+2)       LN normalize -> fp8 (ACT + DVE)
                   stats(t+2..3)   bn_stats/bn_aggr + Newton rsqrt (DVE)

Key changes vs v1 (which ran 364us with PE 54% busy at ~1.2GHz):
  - NO DRAM scratch bounce for the transposed xn: the fp8 pair-view
    (bf16) is transposed SBUF->SBUF through the DMA xbar with a 3D
    output AP (out[a,t,:] = in[:, 128t+a], verified on HW). Saves
    1 MiB/quarter of HBM traffic and two hops in the feed chain.
  - Softmax division moved to the host: the kernel emits the
    unnormalized pooled numerator (f32 PSUM) and the 4 per-quarter
    exp-sum partials per batch; kernel() divides after the gather.
    Kills the per-batch z-bounce/reciprocal/scale serial tail.
  - Quarter-granular pipelining removes the per-batch PE gaps so the
    PE clock can ramp; PE work is ~16.4k cycles/quarter.
Host-side prep folds ln_gamma into W1 and ln_beta@W1+b1 into c2
(b2 dropped: softmax shift-invariance).
"""
import sys
import os

sys.path.insert(0, '/opt/trn_rl_repo')

import numpy as np

import concourse.bass as bass
import concourse.tile as tile
from concourse import bacc, mybir
from concourse.bass_utils import run_bass_kernel_spmd

P = 128
D = 1024
S = 2048
B = 32
NCORES = 8
BLOC = B // NCORES            # batches per core
ROWS = BLOC * S               # 8192 rows per core
SUBT = S // P                 # 16 subtiles per batch
NSUB = 4                      # subtiles per quarter
QS = NSUB * P                 # 512 rows per quarter
NQ = ROWS // QS               # 16 quarters per core
NQB = S // QS                 # 4 quarters per batch
CHUNK = QS                    # matmul moving free dim (= quarter rows)
ET = D // P                   # 8 e-tiles
KT = 4                        # fp8 DoubleRow contraction super-tiles
NPT = 4                       # transposed pair partition-tiles

W1SCALE = 32.0                # host scales W1 by this; undone in tanh's scale

f32 = mybir.dt.float32
f32r = mybir.dt.float32r
bf16 = mybir.dt.bfloat16
fp8 = mybir.dt.float8e4
AF = mybir.ActivationFunctionType
ALU = mybir.AluOpType
DR = mybir.MatmulPerfMode.DoubleRow


def build_nc():
    nc = bacc.Bacc("TRN2", target_bir_lowering=False, num_devices=NCORES)

    x = nc.dram_tensor("x", [ROWS, D], f32, kind="ExternalInput")
    w1p = nc.dram_tensor("w1p", [P, KT, 2, ET, P], fp8, kind="ExternalInput")
    c2v = nc.dram_tensor("c2v", [D], f32, kind="ExternalInput")
    w2v = nc.dram_tensor("w2v", [D], bf16, kind="ExternalInput")
    outn = nc.dram_tensor("outn", [BLOC, D], f32, kind="ExternalOutput")
    outz = nc.dram_tensor("outz", [BLOC, NQB], f32, kind="ExternalOutput")

    with tile.TileContext(nc) as tc:
        with (
            tc.tile_pool(name="consts", bufs=1) as consts,
            tc.tile_pool(name="xa", bufs=6) as xap,        # [128,4,1024] bf16
            tc.tile_pool(name="xn", bufs=3) as xnp,        # [128,4,1024] fp8
            tc.tile_pool(name="xt", bufs=3) as xtp,        # [128,4,1024] fp8
            tc.tile_pool(name="ht", bufs=3) as htp,        # [128,8,512] bf16
            tc.tile_pool(name="stats", bufs=4) as statp,
            tc.tile_pool(name="sc", bufs=2) as scp,
            tc.tile_pool(name="ob", bufs=2) as obp,
            tc.tile_pool(name="psmm", bufs=4, space="PSUM") as psmm,
            tc.tile_pool(name="pssc", bufs=1, space="PSUM") as pssc,
            tc.tile_pool(name="pspl", bufs=2, space="PSUM") as pspl,
            tc.tile_pool(name="dram", bufs=4, space="DRAM") as dramp,
        ):
            # ---- constants ----
            w1_sb = consts.tile(list(w1p.shape), fp8)
            nc.scalar.dma_start(w1_sb, w1p.ap())
            c2_sb = consts.tile([P, ET], f32)
            nc.scalar.dma_start(c2_sb, c2v.ap().rearrange("(t p) -> p t", p=P))
            w2_sb = consts.tile([P, ET], bf16)
            nc.scalar.dma_start(w2_sb, w2v.ap().rearrange("(t p) -> p t", p=P))
            x3 = x.ap().rearrange("(b t p) d -> b t p d", b=BLOC, p=P)

            # per-quarter / per-batch live state
            XA = {}   # q -> x bf16 tile (cast-DMA load)
            XT = {}   # q -> transposed fp8 tile
            XN = {}   # q -> normalized fp8 tile (pre-transpose)
            ST = {}   # q -> (mv, y, mb)
            HT = {}   # q -> tanh output tile
            EPK = {}  # q -> pooling weight tile [P, NSUB] bf16
            SCPS = {}  # b -> scores PSUM tile
            ZC = {}    # b -> exp-sum partials [P, 1]
            EC = {}    # b -> exp rows [P, CHUNK]
            PL = {}    # b -> (pl0, pl1) pooling PSUM tiles

            def emit_load(q, split=False):
                b, c = divmod(q, NQB)
                t0 = NSUB * c
                xa = xap.tile([P, NSUB, D], bf16, tag="xa", name=f"xa{q}")
                XA[q] = xa
                if split:
                    for s2 in range(0, NSUB, 2):
                        nc.gpsimd.dma_start(
                            xa[:, s2:s2 + 2, :],
                            x3[b, t0 + s2:t0 + s2 + 2].rearrange(
                                "t p d -> p t d"))
                else:
                    nc.gpsimd.dma_start(
                        xa, x3[b, t0:t0 + NSUB].rearrange("t p d -> p t d"))

            def emit_stats(q):
                """LN stats + Newton rsqrt for quarter q (DVE)."""
                xaf = XA[q]
                mv = statp.tile([P, NSUB, 2], f32, tag="mv")
                for s in range(NSUB):
                    st = statp.tile([P, 2, 6], f32, tag="bnst")
                    nc.vector.bn_stats(st[:, 0, :], xaf[:, s, 0:512])
                    nc.vector.bn_stats(st[:, 1, :], xaf[:, s, 512:1024])
                    nc.vector.bn_aggr(mv[:, s, :], st)
                # rstd = rsqrt(var+eps): quake seed + 2 Newton steps
                var = statp.tile([P, NSUB], f32, tag="var")
                nc.vector.tensor_scalar(out=var, in0=mv[:, :, 1],
                                        scalar1=1e-5, scalar2=0.5,
                                        op0=ALU.add, op1=ALU.mult)
                y = statp.tile([P, NSUB], f32, tag="y")
                yi = y.bitcast(mybir.dt.int32)
                vi = var.bitcast(mybir.dt.int32)
                nc.vector.tensor_scalar(out=yi, in0=vi, scalar1=0x800000,
                                        scalar2=None, op0=ALU.add)
                nc.vector.tensor_scalar(out=yi, in0=yi, scalar1=1,
                                        scalar2=None,
                                        op0=ALU.logical_shift_right)
                nc.vector.tensor_scalar(out=yi, in0=yi, scalar1=-1,
                                        scalar2=0x5f3759df,
                                        op0=ALU.mult, op1=ALU.add)
                tny = statp.tile([P, NSUB], f32, tag="tny")
                for _ in range(2):
                    nc.vector.tensor_tensor(tny, y, y, ALU.mult)
                    nc.vector.tensor_tensor(tny, tny, var, ALU.mult)
                    nc.vector.tensor_scalar(out=tny, in0=tny, scalar1=-1.0,
                                            scalar2=1.5,
                                            op0=ALU.mult, op1=ALU.add)
                    nc.vector.tensor_tensor(y, y, tny, ALU.mult)
                ST[q] = (mv, y, None)

            def emit_norm(q):
                """Normalize quarter q -> fp8 (all DVE; bf16 in = 2x rate)."""
                mv, y, mb = ST[q]
                xaf = XA[q]
                xnb = xnp.tile([P, NSUB, D], fp8, tag="xn")
                XN[q] = xnb
                for s in range(NSUB):
                    nc.vector.tensor_scalar(out=xnb[:, s, :],
                                            in0=xaf[:, s, :],
                                            scalar1=mv[:, s, 0:1],
                                            scalar2=y[:, s:s + 1],
                                            op0=ALU.subtract,
                                            op1=ALU.mult)

            def emit_transpose(q):
                """SBUF->SBUF xbar transpose of quarter q's fp8 pairs.

                in  (pair view) [128(s), 512(dpair)] per subtile
                out [128(dpair_lo), 4(ptile), 128(s)]:
                    out[a, t, c] = in[c, 128 t + a]   (verified on HW)
                """
                xtt = xtp.tile([P, NPT, D], fp8, tag="xt", name=f"xt{q}")
                XT[q] = xtt
                src = XN[q].bitcast(bf16)        # [P, NSUB, 512]
                dst = xtt.bitcast(bf16)          # [P, NPT, 512]
                for s in range(NSUB):
                    nc.sync.dma_start_transpose(
                        dst[:, :, s * P:(s + 1) * P], src[:, s, :])

            def emit_sc(q, e):
                b, c = divmod(q, NQB)
                nc.tensor.matmul(SCPS[b][32 * c:32 * c + 1, :],
                                 w2_sb[:, e:e + 1], HT[q][:, e, :],
                                 start=(e == 0), stop=(e == ET - 1),
                                 tile_position=(0, 32 * c))

            def emit_p3(q):
                """mm1 (fp8 DoubleRow) + tanh + scores + exp for quarter q."""
                b, c = divmod(q, NQB)
                if c == 0:
                    SCPS[b] = pssc.tile([P, CHUNK], f32, tag="pssc", name=f"scps{b}")
                    ZC[b] = scp.tile([P, 1], f32, tag="zc", name=f"zc{b}")
                    EC[b] = scp.tile([P, CHUNK], bf16, tag="ec", name=f"ec{b}")
                ht = htp.tile([P, ET, CHUNK], bf16, tag="ht", name=f"ht{q}")
                HT[q] = ht
                f8 = XT[q]
                for e in range(ET):
                    ps = psmm.tile([P, CHUNK], f32, tag="mm")
                    for t in range(KT):
                        rhs = f8[:, t, :].rearrange("p (s two) -> p two s",
                                                    two=2)
                        nc.tensor.matmul(ps, w1_sb[:, t, :, e, :], rhs,
                                         start=(t == 0), stop=(t == KT - 1),
                                         perf_mode=DR)
                    nc.scalar.activation(ht[:, e, :], ps, AF.Tanh,
                                         bias=c2_sb[:, e:e + 1],
                                         scale=1.0 / W1SCALE)
                    if e >= 2:
                        emit_sc(q, e - 2)
                emit_sc(q, ET - 2)
                emit_sc(q, ET - 1)
                # exp of this quarter's scores; Z partial via ACT accumulator
                nc.scalar.activation(EC[b][32 * c:32 * c + 1, :],
                                     SCPS[b][32 * c:32 * c + 1, :], AF.Exp,
                                     accum_out=ZC[b][32 * c:32 * c + 1, :])
                # scatter exp row -> [128, NSUB] pooling weights (DRAM bounce)
                eb = dramp.tile([CHUNK], bf16, tag="eb", name=f"eb{q}")
                nc.scalar.dma_start(eb, EC[b][32 * c:32 * c + 1, :])
                epk = scp.tile([P, NSUB], bf16, tag="epk", name=f"epk{q}")
                EPK[q] = epk
                nc.sync.dma_start(epk, eb.rearrange("(t p) -> p t", p=P))

            def emit_pool(q):
                """bf16 pooling matmuls for quarter q (PSUM accum per batch)."""
                b, c = divmod(q, NQB)
                if c == 0:
                    pl0 = pspl.tile([1, 512], f32, tag="pspl", name=f"pl0_{b}")
                    pl1 = pspl.tile([1, 512], f32, tag="pspl", name=f"pl1_{b}")
                    PL[b] = (pl0, pl1)
                pl0, pl1 = PL[b]
                epk = EPK[q]
                xa = XA[q]
                for s in range(NSUB):
                    st = (c == 0 and s == 0)
                    sp = (c == NQB - 1 and s == NSUB - 1)
                    nc.tensor.matmul(pl0, epk[:, s:s + 1], xa[:, s, 0:512],
                                     start=st, stop=sp)
                    nc.tensor.matmul(pl1, epk[:, s:s + 1], xa[:, s, 512:1024],
                                     start=st, stop=sp)

            def emit_tail(b):
                """Unnormalized numerator + Z partials out (host divides)."""
                pl0, pl1 = PL[b]
                ob0 = obp.tile([1, 512], f32, tag="ob0")
                nc.scalar.activation(ob0, pl0, AF.Identity)
                nc.sync.dma_start(outn.ap()[b:b + 1, 0:512], ob0)
                ob1 = obp.tile([1, 512], f32, tag="ob1")
                nc.vector.tensor_copy(ob1, pl1)
                nc.sync.dma_start(outn.ap()[b:b + 1, 512:1024], ob1)
                nc.sync.dma_start(
                    outz.ap()[b:b + 1, :].rearrange("one z -> (one z)"),
                    ZC[b].rearrange("(a b) f -> a (b f)", b=32)[:, 0:1]
                    .rearrange("a one -> (a one)"))

            # ---- static schedule ----
            # prologue: fill the pipe (step t emits norm/transp(t+2), so
            # quarters 0 and 1 are fully prepped here)
            emit_load(0, split=True)
            emit_load(1, split=True)
            emit_stats(0)
            emit_norm(0)
            emit_transpose(0)
            emit_load(2)
            emit_stats(1)
            emit_norm(1)
            emit_transpose(1)
            emit_load(3)
            emit_stats(2)

            for t in range(NQ):
                if t + 4 < NQ:
                    emit_load(t + 4)
                if t + 2 < NQ:
                    emit_norm(t + 2)
                    emit_transpose(t + 2)
                if t - 1 >= 0:
                    emit_pool(t - 1)
                emit_p3(t)
                if t + 3 < NQ:
                    emit_stats(t + 3)
                if t >= 4 and t % NQB == 0:
                    emit_tail(t // NQB - 1)
            emit_pool(NQ - 1)
            emit_tail(BLOC - 1)

    nc.compile()
    return nc


_NC_CACHE = {}


def _get_nc():
    if "nc" not in _NC_CACHE:
        _NC_CACHE["nc"] = build_nc()
    return _NC_CACHE["nc"]


def _prep_host(ln_gamma, ln_beta, W1, b1, W2, b2):
    import ml_dtypes
    W1g = (np.asarray(ln_gamma, np.float32)[:, None]
           * np.asarray(W1, np.float32))
    # pack rows in DoubleRow (super-tile, partition, plane) order:
    # d = t*256 + p*2 + i  ->  arr[p, t, i, e8, e128]
    W1s = (W1g * W1SCALE).astype(ml_dtypes.float8_e4m3)
    W1pk = np.ascontiguousarray(
        W1s.reshape(KT, P, 2, ET, P).transpose(1, 0, 2, 3, 4))
    c2 = (np.asarray(ln_beta, np.float32) @ np.asarray(W1, np.float32)
          + np.asarray(b1, np.float32))
    w2v = np.ascontiguousarray(
        np.asarray(W2, np.float32)[:, 0]).astype(ml_dtypes.bfloat16)
    return W1pk, np.ascontiguousarray(c2), w2v


def run_cores(inputs, trace=False, **kw):
    x = np.asarray(inputs["x"], np.float32)
    W1pk, c2, w2v = _prep_host(inputs["ln_gamma"], inputs["ln_beta"],
                               inputs["W1"], inputs["b1"],
                               inputs["W2"], inputs["b2"])
    nc = _get_nc()
    in_maps = []
    for c in range(NCORES):
        shard = np.ascontiguousarray(
            x[c * BLOC:(c + 1) * BLOC].reshape(ROWS, D))
        in_maps.append(dict(x=shard, w1p=W1pk, c2v=c2, w2v=w2v))
    res = run_bass_kernel_spmd(nc, in_maps, core_ids=list(range(NCORES)),
                               trace=trace, **kw)
    nums = []
    for c in range(NCORES):
        num = np.asarray(res.results[c]["outn"], np.float64)
        z = np.asarray(res.results[c]["outz"], np.float64).sum(axis=1)
        nums.append(num / z[:, None])
    full = np.concatenate(nums, axis=0)
    return full.astype(np.float32), res


def kernel(**inputs) -> np.ndarray:
    out, _ = run_cores(inputs, trace=False)
    return out.astype(np.float32)
